# revision 16
# baseline (speedup 1.0000x reference)
"""Trainium2 Bass kernel for CalculateSLayer GNN message passing.

Computes, for adj [L, L, 2] f32 and h [L, D] f32 with A = adj.sum(-1):
    h_in[j, d]  = sum_i A[i, j] * h[i, d]   (= A.T @ h)
    h_out[i, d] = sum_j A[i, j] * h[j, d]   (= A @ h)

Sharding: core m holds rows i in [m*512, (m+1)*512) of A (for h_in) and
columns j in the same range (for h_out). Both outputs are computed as
per-core partials over the full [L, D] plane and summed on the host.

Wire format: each adjacency channel is centered (a - 0.5) and shipped as
fp8 e4m3 in two layouts (i-major and j-major), pre-tiled per 512-wide
window so every DMA row is one contiguous 4KB run. h is shipped as fp8
plus an fp8 residual (h - fp8(h)), duplicated across the 2 edge-channel
planes. The device runs DoubleRow fp8 matmuls with the h-chunks as the
stationary operand and the adjacency window tiles as the 512-wide moving
operand; the two K-halves are the two edge channels, so the PE performs
the edge-channel sum inside the contraction, and the h8/r8 residual pair
accumulates into the same PSUM tile (no separate fold pass). Outputs
leave the chip d-major (transposed); the host transposes back, folds the
d-tail halves, and undoes the centering by adding colsum(h) (exact, f64)
to every output row.
"""

import numpy as np
import ml_dtypes

L = 4096
D = 150
DD = 2 * D  # [h8 | r8] width
HBW = 512  # padded h-block plane width (ldweights stride alignment)
DT0 = 128  # main d-tile
DT1 = D - DT0  # 22-wide d-tail
NCORES = 8
R = L // NCORES  # 512 rows/cols per core
P = 128  # partitions
IC = R // P  # 4 local chunks per core
JW = 512  # window width along the global axis
NW = L // JW  # 8 windows
TPW = JW // P  # 4 output tiles per window

F8 = ml_dtypes.float8_e4m3

_NC_CACHE = {}
LAST_RESULTS = None


def _ensure_ntff_hook():
    """Register the axon NTFF profile hook if the image's antenv lacks it."""
    import sys
    import types

    try:
        from antenv.axon_hooks import get_axon_ntff_profile_hook  # noqa: F401

        return
    except ImportError:
        pass

    mod = types.ModuleType("antenv.axon_hooks")
    _state = {"hook": None}
    mod.set_axon_ntff_profile_hook = lambda h: _state.__setitem__("hook", h)
    mod.get_axon_ntff_profile_hook = lambda: _state["hook"]
    sys.modules["antenv.axon_hooks"] = mod
    import antenv

    antenv.axon_hooks = mod

    so_path = "/opt/axon/libaxon_pjrt.so"
    try:
        from trn_agent_boot.trn_boot import _ntff_profile_via_ctypes

        hook = _ntff_profile_via_ctypes(so_path)
        if hook is not None:
            mod.set_axon_ntff_profile_hook(hook)
    except Exception:
        pass

    try:
        from concourse import bass_utils

        bass_utils.upload_artifacts = lambda tmpdir: tmpdir
    except Exception:
        pass


def _build_nc():
    import concourse.bacc as bacc
    import concourse.tile as tile
    import concourse.mybir as mybir

    f8 = mybir.dt.float8e4
    f32 = mybir.dt.float32
    bf16 = mybir.dt.bfloat16
    DR = mybir.MatmulPerfMode.DoubleRow

    nc = bacc.Bacc(
        "TRN2", target_bir_lowering=False, debug=False, num_devices=NCORES
    )
    # centered fp8 channels, pre-windowed: [P, window, chunk, channel, JW]
    aw_d = nc.dram_tensor("aw", [P, NW, IC, 2, JW], f8, kind="ExternalInput").ap()
    atw_d = nc.dram_tensor("atw", [P, NW, IC, 2, JW], f8,
                           kind="ExternalInput").ap()
    # this core's h rows as [h8 | r8], duplicated over the channel axis
    hb_d = nc.dram_tensor("hbdup", [P, IC, 2, HBW], f8, kind="ExternalInput").ap()
    # outputs: j-major partials, partition-major for contiguous DMA rows
    pin_d = nc.dram_tensor("pin", [P, NW, TPW, D], bf16,
                           kind="ExternalOutput").ap()
    hout_d = nc.dram_tensor("hout", [P, NW, TPW, D], bf16,
                            kind="ExternalOutput").ap()

    with tile.TileContext(nc) as tc:
        with (
            tc.tile_pool(name="const", bufs=1) as const_pool,
            tc.tile_pool(name="adj", bufs=3) as adj_pool,
            tc.tile_pool(name="stage", bufs=2) as stage_pool,
            tc.tile_pool(name="ps", bufs=2, space="PSUM") as psum_pool,
        ):
            # moving operand: [h8 d0:150 | r8 d0:150] per chunk/channel
            # (issued first on the sync queue: every matmul needs it)
            hb_sb = const_pool.tile([P, IC, 2, HBW], f8)
            nc.sync.dma_start(hb_sb[:], hb_d)

            outs = (("pin", pin_d), ("po", hout_d))

            for w in range(NW):
                a_sb = adj_pool.tile([P, IC, 2, JW], f8, tag="a", name="a_sb")
                at_sb = adj_pool.tile([P, IC, 2, JW], f8, tag="at",
                                      name="at_sb")
                if w == 0:
                    # split the first loads so matmuls start sooner
                    for k in range(IC):
                        nc.sync.dma_start(a_sb[:, k], aw_d[:, w, k])
                    for k in range(IC):
                        nc.scalar.dma_start(at_sb[:, k], atw_d[:, w, k])
                else:
                    nc.sync.dma_start(a_sb[:], aw_d[:, w])
                    nc.sync.dma_start(at_sb[:], atw_d[:, w])

                for g, (gname, out_d) in enumerate(outs):
                    mov = a_sb if g == 0 else at_sb
                    pb = psum_pool.tile([P, TPW, JW], f32, tag=f"pb{gname}",
                                        name=f"pb{gname}", bufs=1)
                    for t in range(TPW):
                        for k in range(IC):
                            nc.tensor.matmul(
                                pb[:, t, 0:DD],
                                mov[:, k, :, t * P : (t + 1) * P],
                                hb_sb[:, k, :, 0:DD],
                                start=(k == 0),
                                stop=(k == IC - 1),
                                perf_mode=DR,
                            )
                    # fold the r8 half into the h8 half: one PSUM operand
                    # per instruction (scalar stages r8, vector adds+casts);
                    # per-t so folds start before the whole group finishes
                    rs = stage_pool.tile([P, TPW, D], f32, tag=f"rs{gname}",
                                         name=f"rs{gname}")
                    st = stage_pool.tile([P, TPW, D], bf16, tag=f"st{gname}",
                                         name=f"st{gname}")
                    for t in range(TPW):
                        nc.scalar.copy(rs[:, t, :], pb[:, t, D:DD])
                        nc.vector.tensor_add(
                            st[:, t, :], pb[:, t, 0:D], rs[:, t, :]
                        )
                    nc.gpsimd.dma_start(out_d[:, w], st[:])

    nc.compile()
    return nc


def _get_nc():
    if "nc" not in _NC_CACHE:
        _NC_CACHE["nc"] = _build_nc()
    return _NC_CACHE["nc"]


def _prep_inputs(adj, h):
    """Quantize + shard on the host; returns per-core input dicts."""
    b8 = (adj - np.float32(0.5)).astype(F8)  # [L, L, 2] centered channels
    h8 = h.astype(F8)
    r8 = (h - h8.astype(np.float32)).astype(F8)
    # [h8 d0:128 | r8 d0:128 | h8 d128:150 | r8 d128:150]
    hd = np.zeros((L, HBW), dtype=F8)
    hd[:, 0:D] = h8
    hd[:, D:DD] = r8

    in_maps = []
    for m in range(NCORES):
        rows = b8[m * R : (m + 1) * R]  # [R, L, 2] = [i_local, j, c]
        cols = b8[:, m * R : (m + 1) * R, :]  # [L, R, 2] = [i, j_local, c]
        # [P, NW, IC, 2, JW]
        aw = np.ascontiguousarray(
            rows.reshape(IC, P, NW, JW, 2).transpose(1, 2, 0, 4, 3)
        )
        atw = np.ascontiguousarray(
            cols.transpose(1, 2, 0).reshape(IC, P, 2, NW, JW)
            .transpose(1, 3, 0, 2, 4)
        )
        blk = hd[m * R : (m + 1) * R].reshape(IC, P, HBW).transpose(1, 0, 2)
        hbdup = np.ascontiguousarray(
            np.broadcast_to(blk[:, :, None, :], (P, IC, 2, HBW))
        )
        in_maps.append({"aw": aw, "atw": atw, "hbdup": hbdup})
    return in_maps


def _run_cores(adj, h, trace=False):
    from concourse.bass_utils import run_bass_kernel_spmd

    global LAST_RESULTS
    if trace:
        _ensure_ntff_hook()
    nc = _get_nc()
    in_maps = _prep_inputs(adj, h)
    res = run_bass_kernel_spmd(
        nc, in_maps, core_ids=list(range(NCORES)), trace=trace
    )
    LAST_RESULTS = res
    return res


def kernel(unpreprocessed_unweight_adj_matrix, h):
    adj = np.ascontiguousarray(
        np.asarray(unpreprocessed_unweight_adj_matrix, dtype=np.float32)
    )
    h = np.ascontiguousarray(np.asarray(h, dtype=np.float32))
    res = _run_cores(adj, h)
    parts = res.results

    colsum = h.astype(np.float64).sum(axis=0)  # undo the -0.5 centering
    h_in = np.zeros((L, D), dtype=np.float64)
    h_out = np.zeros((L, D), dtype=np.float64)
    for r in parts:
        for acc, key in ((h_in, "pin"), (h_out, "hout")):
            p = np.asarray(r[key], dtype=np.float32)  # [P, NW, TPW, D]
            acc += p.transpose(1, 2, 0, 3).reshape(L, D)
    h_in += colsum[None, :]
    h_out += colsum[None, :]
    return (
        np.ascontiguousarray(h_in, dtype=np.float32),
        np.ascontiguousarray(h_out, dtype=np.float32),
    )


# revision 17
# speedup vs baseline: 1.3786x; 1.3786x over previous
"""Trainium2 Bass kernel for CalculateSLayer GNN message passing.

Computes, for adj [L, L, 2] f32 and h [L, D] f32 with A = adj.sum(-1):
    h_in[j, d]  = sum_i A[i, j] * h[i, d]   (= A.T @ h)
    h_out[i, d] = sum_j A[i, j] * h[j, d]   (= A @ h)

Sharding: core m holds rows i in [m*512, (m+1)*512) of A (for h_in) and
columns j in the same range (for h_out). Both outputs are computed as
per-core partials over the full [L, D] plane and summed on the host.

Wire format: each adjacency channel is centered (a - 0.5) and shipped as
fp8 e4m3 in two layouts (i-major and j-major), pre-tiled per 512-wide
window so every DMA row is one contiguous 4KB run. h is shipped as fp8
plus an fp8 residual (h - fp8(h)), duplicated across the 2 edge-channel
planes. The device runs DoubleRow fp8 matmuls with the h-chunks as the
stationary operand and the adjacency window tiles as the 512-wide moving
operand; the two K-halves are the two edge channels, so the PE performs
the edge-channel sum inside the contraction, and the h8/r8 residual pair
accumulates into the same PSUM tile (no separate fold pass). Outputs
leave the chip d-major (transposed); the host transposes back, folds the
d-tail halves, and undoes the centering by adding colsum(h) (exact, f64)
to every output row.
"""

import numpy as np
import ml_dtypes

L = 4096
D = 150
DD = 2 * D  # [h8 | r8] width
HBW = 512  # padded h-block plane width (ldweights stride alignment)
DT0 = 128  # main d-tile
DT1 = D - DT0  # 22-wide d-tail
NCORES = 8
R = L // NCORES  # 512 rows/cols per core
P = 128  # partitions
IC = R // P  # 4 local chunks per core
JW = 512  # window width along the global axis
NW = L // JW  # 8 windows
TPW = JW // P  # 4 output tiles per window

F8 = ml_dtypes.float8_e4m3

_NC_CACHE = {}
LAST_RESULTS = None


def _ensure_ntff_hook():
    """Register the axon NTFF profile hook if the image's antenv lacks it."""
    import sys
    import types

    try:
        from antenv.axon_hooks import get_axon_ntff_profile_hook  # noqa: F401

        return
    except ImportError:
        pass

    mod = types.ModuleType("antenv.axon_hooks")
    _state = {"hook": None}
    mod.set_axon_ntff_profile_hook = lambda h: _state.__setitem__("hook", h)
    mod.get_axon_ntff_profile_hook = lambda: _state["hook"]
    sys.modules["antenv.axon_hooks"] = mod
    import antenv

    antenv.axon_hooks = mod

    so_path = "/opt/axon/libaxon_pjrt.so"
    try:
        from trn_agent_boot.trn_boot import _ntff_profile_via_ctypes

        hook = _ntff_profile_via_ctypes(so_path)
        if hook is not None:
            mod.set_axon_ntff_profile_hook(hook)
    except Exception:
        pass

    try:
        from concourse import bass_utils

        bass_utils.upload_artifacts = lambda tmpdir: tmpdir
    except Exception:
        pass


def _build_nc():
    import concourse.bacc as bacc
    import concourse.tile as tile
    import concourse.mybir as mybir

    f8 = mybir.dt.float8e4
    f32 = mybir.dt.float32
    bf16 = mybir.dt.bfloat16
    DR = mybir.MatmulPerfMode.DoubleRow

    nc = bacc.Bacc(
        "TRN2", target_bir_lowering=False, debug=False, num_devices=NCORES
    )
    # centered fp8 channels, pre-windowed: [P, window, chunk, channel, JW]
    aw_d = nc.dram_tensor("aw", [P, NW, IC, 2, JW], f8, kind="ExternalInput").ap()
    atw_d = nc.dram_tensor("atw", [P, NW, IC, 2, JW], f8,
                           kind="ExternalInput").ap()
    # this core's h rows as [h8 | r8], duplicated over the channel axis
    hb_d = nc.dram_tensor("hbdup", [P, IC, 2, HBW], f8, kind="ExternalInput").ap()
    # outputs: j-major partials, partition-major for contiguous DMA rows
    pin_d = nc.dram_tensor("pin", [P, NW, TPW, D], bf16,
                           kind="ExternalOutput").ap()
    hout_d = nc.dram_tensor("hout", [P, NW, TPW, D], bf16,
                            kind="ExternalOutput").ap()

    with tile.TileContext(nc) as tc:
        with (
            tc.tile_pool(name="const", bufs=1) as const_pool,
            tc.tile_pool(name="adj", bufs=3) as adj_pool,
            tc.tile_pool(name="stage", bufs=2) as stage_pool,
            tc.tile_pool(name="ps", bufs=2, space="PSUM") as psum_pool,
        ):
            # moving operand: [h8 d0:150 | r8 d0:150] per chunk/channel
            # (issued first on the sync queue: every matmul needs it)
            hb_sb = const_pool.tile([P, IC, 2, HBW], f8)
            nc.sync.dma_start(hb_sb[:], hb_d)

            outs = (("pin", pin_d), ("po", hout_d))

            for w in range(NW):
                a_sb = adj_pool.tile([P, IC, 2, JW], f8, tag="a", name="a_sb")
                at_sb = adj_pool.tile([P, IC, 2, JW], f8, tag="at",
                                      name="at_sb")
                if w == 0:
                    # split the first loads so matmuls start sooner
                    for k in range(IC):
                        nc.sync.dma_start(a_sb[:, k], aw_d[:, w, k])
                    for k in range(IC):
                        nc.scalar.dma_start(at_sb[:, k], atw_d[:, w, k])
                else:
                    nc.sync.dma_start(a_sb[:], aw_d[:, w])
                    nc.sync.dma_start(at_sb[:], atw_d[:, w])

                for g, (gname, out_d) in enumerate(outs):
                    mov = a_sb if g == 0 else at_sb
                    st = stage_pool.tile([P, TPW, D], bf16, tag=f"st{gname}",
                                         name=f"st{gname}")
                    # two 2-bank PSUM half-groups per gemm: the eviction of
                    # one half overlaps the other halves' matmuls, so the
                    # next window never waits on a fold chain
                    for hx in range(2):
                        pb = psum_pool.tile([P, 2, JW], f32,
                                            tag=f"pb{gname}{hx}",
                                            name=f"pb{gname}{hx}", bufs=1)
                        for ti in range(2):
                            t = hx * 2 + ti
                            for k in range(IC):
                                nc.tensor.matmul(
                                    pb[:, ti, 0:DD],
                                    mov[:, k, :, t * P : (t + 1) * P],
                                    hb_sb[:, k, :, 0:DD],
                                    start=(k == 0),
                                    stop=(k == IC - 1),
                                    perf_mode=DR,
                                )
                        # fold r8 into h8: scalar stages r8 (one PSUM
                        # operand per instruction), vector adds + casts
                        rs = stage_pool.tile([P, 2, D], f32,
                                             tag=f"rs{gname}{hx}",
                                             name=f"rs{gname}{hx}")
                        nc.scalar.copy(rs[:], pb[:, :, D:DD])
                        nc.vector.tensor_add(
                            st[:, hx * 2 : hx * 2 + 2, :],
                            pb[:, :, 0:D],
                            rs[:],
                        )
                    nc.gpsimd.dma_start(out_d[:, w], st[:])

    nc.compile()
    return nc


def _get_nc():
    if "nc" not in _NC_CACHE:
        _NC_CACHE["nc"] = _build_nc()
    return _NC_CACHE["nc"]


def _prep_inputs(adj, h):
    """Quantize + shard on the host; returns per-core input dicts."""
    b8 = (adj - np.float32(0.5)).astype(F8)  # [L, L, 2] centered channels
    h8 = h.astype(F8)
    r8 = (h - h8.astype(np.float32)).astype(F8)
    # [h8 d0:128 | r8 d0:128 | h8 d128:150 | r8 d128:150]
    hd = np.zeros((L, HBW), dtype=F8)
    hd[:, 0:D] = h8
    hd[:, D:DD] = r8

    in_maps = []
    for m in range(NCORES):
        rows = b8[m * R : (m + 1) * R]  # [R, L, 2] = [i_local, j, c]
        cols = b8[:, m * R : (m + 1) * R, :]  # [L, R, 2] = [i, j_local, c]
        # [P, NW, IC, 2, JW]
        aw = np.ascontiguousarray(
            rows.reshape(IC, P, NW, JW, 2).transpose(1, 2, 0, 4, 3)
        )
        atw = np.ascontiguousarray(
            cols.transpose(1, 2, 0).reshape(IC, P, 2, NW, JW)
            .transpose(1, 3, 0, 2, 4)
        )
        blk = hd[m * R : (m + 1) * R].reshape(IC, P, HBW).transpose(1, 0, 2)
        hbdup = np.ascontiguousarray(
            np.broadcast_to(blk[:, :, None, :], (P, IC, 2, HBW))
        )
        in_maps.append({"aw": aw, "atw": atw, "hbdup": hbdup})
    return in_maps


def _run_cores(adj, h, trace=False):
    from concourse.bass_utils import run_bass_kernel_spmd

    global LAST_RESULTS
    if trace:
        _ensure_ntff_hook()
    nc = _get_nc()
    in_maps = _prep_inputs(adj, h)
    res = run_bass_kernel_spmd(
        nc, in_maps, core_ids=list(range(NCORES)), trace=trace
    )
    LAST_RESULTS = res
    return res


def kernel(unpreprocessed_unweight_adj_matrix, h):
    adj = np.ascontiguousarray(
        np.asarray(unpreprocessed_unweight_adj_matrix, dtype=np.float32)
    )
    h = np.ascontiguousarray(np.asarray(h, dtype=np.float32))
    res = _run_cores(adj, h)
    parts = res.results

    colsum = h.astype(np.float64).sum(axis=0)  # undo the -0.5 centering
    h_in = np.zeros((L, D), dtype=np.float64)
    h_out = np.zeros((L, D), dtype=np.float64)
    for r in parts:
        for acc, key in ((h_in, "pin"), (h_out, "hout")):
            p = np.asarray(r[key], dtype=np.float32)  # [P, NW, TPW, D]
            acc += p.transpose(1, 2, 0, 3).reshape(L, D)
    h_in += colsum[None, :]
    h_out += colsum[None, :]
    return (
        np.ascontiguousarray(h_in, dtype=np.float32),
        np.ascontiguousarray(h_out, dtype=np.float32),
    )


# revision 19
# speedup vs baseline: 1.4342x; 1.0403x over previous
"""Trainium2 Bass kernel for CalculateSLayer GNN message passing.

Computes, for adj [L, L, 2] f32 and h [L, D] f32 with A = adj.sum(-1):
    h_in[j, d]  = sum_i A[i, j] * h[i, d]   (= A.T @ h)
    h_out[i, d] = sum_j A[i, j] * h[j, d]   (= A @ h)

Sharding: core m holds rows i in [m*512, (m+1)*512) of A (for h_in) and
columns j in the same range (for h_out). Both outputs are computed as
per-core partials over the full [L, D] plane and summed on the host.

Wire format: each adjacency channel is centered (a - 0.5) and shipped as
fp8 e4m3 in two layouts (i-major and j-major), pre-tiled per 512-wide
window so every DMA row is one contiguous 4KB run. h is shipped as fp8
plus an fp8 residual (h - fp8(h)), duplicated across the 2 edge-channel
planes. The device runs DoubleRow fp8 matmuls with the h-chunks as the
stationary operand and the adjacency window tiles as the 512-wide moving
operand; the two K-halves are the two edge channels, so the PE performs
the edge-channel sum inside the contraction, and the h8/r8 residual pair
accumulates into the same PSUM tile (no separate fold pass). Outputs
leave the chip d-major (transposed); the host transposes back, folds the
d-tail halves, and undoes the centering by adding colsum(h) (exact, f64)
to every output row.
"""

import numpy as np
import ml_dtypes

L = 4096
D = 150
DD = 2 * D  # [h8 | r8] width
HBW = 300  # h-block plane width ([h8 | r8], moving operand)
DT0 = 128  # main d-tile
DT1 = D - DT0  # 22-wide d-tail
NCORES = 8
R = L // NCORES  # 512 rows/cols per core
P = 128  # partitions
IC = R // P  # 4 local chunks per core
JW = 512  # window width along the global axis
NW = L // JW  # 8 windows
TPW = JW // P  # 4 output tiles per window

F8 = ml_dtypes.float8_e4m3

_NC_CACHE = {}
LAST_RESULTS = None


def _ensure_ntff_hook():
    """Register the axon NTFF profile hook if the image's antenv lacks it."""
    import sys
    import types

    try:
        from antenv.axon_hooks import get_axon_ntff_profile_hook  # noqa: F401

        return
    except ImportError:
        pass

    mod = types.ModuleType("antenv.axon_hooks")
    _state = {"hook": None}
    mod.set_axon_ntff_profile_hook = lambda h: _state.__setitem__("hook", h)
    mod.get_axon_ntff_profile_hook = lambda: _state["hook"]
    sys.modules["antenv.axon_hooks"] = mod
    import antenv

    antenv.axon_hooks = mod

    so_path = "/opt/axon/libaxon_pjrt.so"
    try:
        from trn_agent_boot.trn_boot import _ntff_profile_via_ctypes

        hook = _ntff_profile_via_ctypes(so_path)
        if hook is not None:
            mod.set_axon_ntff_profile_hook(hook)
    except Exception:
        pass

    try:
        from concourse import bass_utils

        bass_utils.upload_artifacts = lambda tmpdir: tmpdir
    except Exception:
        pass


def _build_nc():
    import concourse.bacc as bacc
    import concourse.tile as tile
    import concourse.mybir as mybir
    from concourse.masks import make_identity

    f8 = mybir.dt.float8e4
    f32 = mybir.dt.float32
    bf16 = mybir.dt.bfloat16
    DR = mybir.MatmulPerfMode.DoubleRow

    nc = bacc.Bacc(
        "TRN2", target_bir_lowering=False, debug=False, num_devices=NCORES
    )
    # centered fp8 channels, pre-windowed: [P, window, chunk, channel, JW]
    aw_d = nc.dram_tensor("aw", [P, NW, IC, 2, JW], f8, kind="ExternalInput").ap()
    atw_d = nc.dram_tensor("atw", [P, NW, IC, 2, JW], f8,
                           kind="ExternalInput").ap()
    # this core's h rows as [h8 | r8], duplicated over the channel axis
    hb_d = nc.dram_tensor("hbdup", [P, IC, 2, HBW], f8, kind="ExternalInput").ap()
    # outputs: j-major partials, partition-major for contiguous DMA rows
    pin_d = nc.dram_tensor("pin", [P, NW, TPW, D], bf16,
                           kind="ExternalOutput").ap()
    hout_d = nc.dram_tensor("hout", [P, NW, TPW, D], bf16,
                            kind="ExternalOutput").ap()

    with tile.TileContext(nc) as tc:
        with (
            tc.tile_pool(name="const", bufs=1) as const_pool,
            tc.tile_pool(name="adj", bufs=3) as adj_pool,
            tc.tile_pool(name="stage", bufs=2) as stage_pool,
            tc.tile_pool(name="ps", bufs=2, space="PSUM") as psum_pool,
        ):
            # moving operand: [h8 d0:150 | r8 d0:150] per chunk/channel
            # (own queue so it doesn't serialize behind the window loads)
            hb_sb = const_pool.tile([P, IC, 2, HBW], f8)
            nc.scalar.dma_start(hb_sb[:], hb_d)

            # warm the PE p-state with dummy matmuls while DMAs land; the
            # results land in the first pin half-group bank and are
            # overwritten by the real accumulation (start=True zeroes it)
            ident = const_pool.tile([P, P], bf16)
            make_identity(nc, ident[:])
            warm = psum_pool.tile([P, 2, JW], f32, tag="pbpin0",
                                  name="warm", bufs=1)
            for i in range(14):
                nc.tensor.matmul(warm[:, 0, 0:P], ident[:], ident[:])

            outs = (("pin", pin_d), ("po", hout_d))

            for w in range(NW):
                a_sb = adj_pool.tile([P, IC, 2, JW], f8, tag="a", name="a_sb")
                at_sb = adj_pool.tile([P, IC, 2, JW], f8, tag="at",
                                      name="at_sb")
                if w == 0:
                    # split the first loads so matmuls start sooner
                    for k in range(IC):
                        nc.sync.dma_start(a_sb[:, k], aw_d[:, w, k])
                    for k in range(IC):
                        nc.gpsimd.dma_start(at_sb[:, k], atw_d[:, w, k])
                else:
                    nc.sync.dma_start(a_sb[:], aw_d[:, w])
                    nc.sync.dma_start(at_sb[:], atw_d[:, w])

                for g, (gname, out_d) in enumerate(outs):
                    mov = a_sb if g == 0 else at_sb
                    st = stage_pool.tile([P, TPW, D], bf16, tag=f"st{gname}",
                                         name=f"st{gname}")
                    # two 2-bank PSUM half-groups per gemm: the eviction of
                    # one half overlaps the other halves' matmuls, so the
                    # next window never waits on a fold chain
                    for hx in range(2):
                        pb = psum_pool.tile([P, 2, JW], f32,
                                            tag=f"pb{gname}{hx}",
                                            name=f"pb{gname}{hx}", bufs=1)
                        for ti in range(2):
                            t = hx * 2 + ti
                            for k in range(IC):
                                nc.tensor.matmul(
                                    pb[:, ti, 0:DD],
                                    mov[:, k, :, t * P : (t + 1) * P],
                                    hb_sb[:, k, :, 0:DD],
                                    start=(k == 0),
                                    stop=(k == IC - 1),
                                    perf_mode=DR,
                                )
                        # fold r8 into h8: scalar stages r8 (one PSUM
                        # operand per instruction), vector adds + casts
                        rs = stage_pool.tile([P, 2, D], f32,
                                             tag=f"rs{gname}{hx}",
                                             name=f"rs{gname}{hx}")
                        nc.scalar.copy(rs[:], pb[:, :, D:DD])
                        nc.vector.tensor_add(
                            st[:, hx * 2 : hx * 2 + 2, :],
                            pb[:, :, 0:D],
                            rs[:],
                        )
                        eng = nc.gpsimd if hx == 0 else nc.scalar
                        eng.dma_start(
                            out_d[:, w, hx * 2 : hx * 2 + 2],
                            st[:, hx * 2 : hx * 2 + 2, :],
                        )

    nc.compile()
    return nc


def _get_nc():
    if "nc" not in _NC_CACHE:
        _NC_CACHE["nc"] = _build_nc()
    return _NC_CACHE["nc"]


def _prep_inputs(adj, h):
    """Quantize + shard on the host; returns per-core input dicts."""
    b8 = (adj - np.float32(0.5)).astype(F8)  # [L, L, 2] centered channels
    h8 = h.astype(F8)
    r8 = (h - h8.astype(np.float32)).astype(F8)
    # [h8 d0:128 | r8 d0:128 | h8 d128:150 | r8 d128:150]
    hd = np.zeros((L, HBW), dtype=F8)
    hd[:, 0:D] = h8
    hd[:, D:DD] = r8

    in_maps = []
    for m in range(NCORES):
        rows = b8[m * R : (m + 1) * R]  # [R, L, 2] = [i_local, j, c]
        cols = b8[:, m * R : (m + 1) * R, :]  # [L, R, 2] = [i, j_local, c]
        # [P, NW, IC, 2, JW]
        aw = np.ascontiguousarray(
            rows.reshape(IC, P, NW, JW, 2).transpose(1, 2, 0, 4, 3)
        )
        atw = np.ascontiguousarray(
            cols.transpose(1, 2, 0).reshape(IC, P, 2, NW, JW)
            .transpose(1, 3, 0, 2, 4)
        )
        blk = hd[m * R : (m + 1) * R].reshape(IC, P, HBW).transpose(1, 0, 2)
        hbdup = np.ascontiguousarray(
            np.broadcast_to(blk[:, :, None, :], (P, IC, 2, HBW))
        )
        in_maps.append({"aw": aw, "atw": atw, "hbdup": hbdup})
    return in_maps


def _run_cores(adj, h, trace=False):
    from concourse.bass_utils import run_bass_kernel_spmd

    global LAST_RESULTS
    if trace:
        _ensure_ntff_hook()
    nc = _get_nc()
    in_maps = _prep_inputs(adj, h)
    res = run_bass_kernel_spmd(
        nc, in_maps, core_ids=list(range(NCORES)), trace=trace
    )
    LAST_RESULTS = res
    return res


def kernel(unpreprocessed_unweight_adj_matrix, h):
    adj = np.ascontiguousarray(
        np.asarray(unpreprocessed_unweight_adj_matrix, dtype=np.float32)
    )
    h = np.ascontiguousarray(np.asarray(h, dtype=np.float32))
    res = _run_cores(adj, h)
    parts = res.results

    colsum = h.astype(np.float64).sum(axis=0)  # undo the -0.5 centering
    h_in = np.zeros((L, D), dtype=np.float64)
    h_out = np.zeros((L, D), dtype=np.float64)
    for r in parts:
        for acc, key in ((h_in, "pin"), (h_out, "hout")):
            p = np.asarray(r[key], dtype=np.float32)  # [P, NW, TPW, D]
            acc += p.transpose(1, 2, 0, 3).reshape(L, D)
    h_in += colsum[None, :]
    h_out += colsum[None, :]
    return (
        np.ascontiguousarray(h_in, dtype=np.float32),
        np.ascontiguousarray(h_out, dtype=np.float32),
    )
